# revision 38
# baseline (speedup 1.0000x reference)
"""Trainium2 Bass kernel for nn_CausalWanSelfAttention (sparse_attention).

Strategy: tensor-parallel over heads across 8 NeuronCores. Each core owns
2 of the 16 heads and processes all 1760 tokens:
  - fused QKV projection (bf16 matmuls, PSUM-accumulated over 16 k-tiles)
  - RMS-norm: local sum-of-squares, tiny AllReduce for the full-2048-channel
    statistic, ln/exp-based rsqrt on ACT
  - rope on DVE (free-dim channel pairs), PE-transpose of q/k per head
  - attention in transposed layout: scores^T = kw^T(T) @ rq^T, exp on ACT,
    PV accumulation on PE; softmax denominator via DVE accumulation +
    GPSIMD partition_all_reduce; normalize with a DVE divide
  - output projection, f32 ReduceScatter so each core emits 220 rows
Host side (free): input slicing/transposition/bf16 casts, rope freq table,
final concat + output bias.
"""
import sys

for _p in ("/opt/trn_rl_repo", "/root/.axon_site/_ro/trn_rl_repo"):
    if _p not in sys.path:
        sys.path.append(_p)

import numpy as np
import ml_dtypes

import concourse.bass as bass
import concourse.bacc as bacc
import concourse.mybir as mybir
from concourse import bass_isa
from concourse.tile import TileContext
from concourse.bass_utils import run_bass_kernel_spmd
from concourse.masks import make_identity

BF16 = ml_dtypes.bfloat16
S, DIM, NH, D = 1760, 2048, 16, 128
TW = 3520          # attention window length
WIN0 = 2640        # cache rows [2640:4400] form the first half of the window
NCORES, HPC = 8, 2
CH = HPC * D       # 256 channels per core
EPS = 1e-6
SCALE = 1.0 / float(np.sqrt(D))
S_OUT = S // NCORES  # 220 rows of output per core

S_TILES = [(i * 128, min(128, S - i * 128)) for i in range((S + 127) // 128)]
NT = len(S_TILES)  # 14
# window t-tiles: cache part [0,1760) then new part [1760,3520)
T_TILES = ([("c", j, off, sz) for j, (off, sz) in enumerate(S_TILES)]
           + [("n", j, off + S, sz) for j, (off, sz) in enumerate(S_TILES)])
SJ = [(0, 880), (880, 880)]  # attention s-chunks

_CACHE = {}


def _emit(nc):
    dt = mybir.dt
    BF, F32 = dt.bfloat16, dt.float32
    A = mybir.ActivationFunctionType
    Op = mybir.AluOpType
    core_ids = list(range(NCORES))

    xT = nc.declare_dram_parameter("xT", [NT, 128, DIM], BF, isOutput=False)
    wT = nc.declare_dram_parameter("wT", [DIM, 3 * CH], BF, isOutput=False)
    woT = nc.declare_dram_parameter("woT", [DIM, DIM], BF, isOutput=False)
    ckT = nc.declare_dram_parameter("ckT", [HPC, D, S], BF, isOutput=False)
    cv = nc.declare_dram_parameter("cv", [HPC, 128, NT * D], BF, isOutput=False)
    fr2d = nc.declare_dram_parameter("fr2", [S, 2 * 64], BF, isOutput=False)
    fi2d = nc.declare_dram_parameter("fi2", [S, 2 * 64], BF, isOutput=False)
    gqd = nc.declare_dram_parameter("gq", [1, CH], F32, isOutput=False)
    gkd = nc.declare_dram_parameter("gk", [1, CH], F32, isOutput=False)
    bqd = nc.declare_dram_parameter("bq", [1, CH], F32, isOutput=False)
    bkd = nc.declare_dram_parameter("bk", [1, CH], F32, isOutput=False)
    bvd = nc.declare_dram_parameter("bv", [1, CH], F32, isOutput=False)
    y_out = nc.declare_dram_parameter("y", [S_OUT, DIM], F32, isOutput=True)

    ss_in = [nc.dram_tensor(f"ss_in{g}", [2, 128, 7], F32) for g in range(2)]
    ss_out = [nc.dram_tensor(f"ss_out{g}", [2, 128, 7], F32, addr_space="Shared")
              for g in range(2)]
    # o-matrix all-to-all: two waves (s 0:880 and 880:1760) x two heads;
    # each core ends up with o^T columns for its own 110-row slice
    a2a_in = [[nc.dram_tensor(f"a2a_in{w}_{h}", [NCORES, D, 110], BF)
               for h in range(2)] for w in range(2)]
    a2a_out = [[nc.dram_tensor(f"a2a_out{w}_{h}", [NCORES, D, 110], BF)
                for h in range(2)] for w in range(2)]

    from contextlib import ExitStack
    with TileContext(nc) as tc, ExitStack() as stack:
        cpool = stack.enter_context(tc.tile_pool(name="const", bufs=1))
        wpool = stack.enter_context(tc.tile_pool(name="work", bufs=3))
        ppool = tc.alloc_tile_pool(name="projp", bufs=1)

        # ---- constants ----
        ident = cpool.tile([128, 128], BF, tag="ident")
        make_identity(nc, ident[:])
        ones_col = cpool.tile([128, 1], BF, tag="ones_col")
        nc.gpsimd.memset(ones_col[:], 1.0)
        ones128 = cpool.tile([1, 128], BF, tag="ones128")
        nc.gpsimd.memset(ones128[:], 1.0)

        xt0 = wpool.tile([128, DIM], BF, tag="xt0", bufs=1, name="xt0pre")
        nc.sync.dma_start(out=xt0[:], in_=xT[0])
        wT_sb = []
        for kk in range(16):
            t = ppool.tile([128, 3 * CH], BF, tag=f"wT{kk}", name=f"wT{kk}")
            eng = nc.sync if kk % 2 == 0 else nc.scalar
            eng.dma_start(out=t[:], in_=wT[128 * kk:128 * (kk + 1), :])
            wT_sb.append(t)

        def bcast_row(name, src):
            row = cpool.tile([1, CH], F32, tag=f"{name}_row", name=f"{name}_row")
            nc.sync.dma_start(out=row[:], in_=src[:])
            full = cpool.tile([128, CH], F32, tag=f"{name}_full", name=f"{name}_full")
            nc.gpsimd.partition_broadcast(full[:], row[:])
            return full

        def bias_row(name, src):
            # bf16 [1, CH] row used as the rhs of a rank-1 bias matmul
            row = cpool.tile([1, CH], F32, tag=f"{name}_row", name=f"{name}_row")
            nc.sync.dma_start(out=row[:], in_=src[:])
            rbf = cpool.tile([1, CH], BF, tag=f"{name}_bf", name=f"{name}_bf")
            nc.vector.tensor_copy(rbf[:], row[:])
            return rbf

        gqB = bcast_row("gq", gqd)
        gkB = bcast_row("gk", gkd)
        bqR = bias_row("bq", bqd)
        bkR = bias_row("bk", bkd)
        bvR = bias_row("bv", bvd)

        kwT_sb = []
        for hh in range(HPC):
            t = cpool.tile([128, TW], BF, tag=f"kwT{hh}", name=f"kwT{hh}")
            kwT_sb.append(t)
        cv_sb = [[], []]

        q_sb, k_sb, v_sb = [], [], []
        for j in range(NT):
            q_sb.append(ppool.tile([128, CH], F32, tag=f"q{j}", name=f"q{j}"))
            k_sb.append(ppool.tile([128, CH], F32, tag=f"k{j}", name=f"k{j}"))
            v_sb.append(cpool.tile([128, CH], BF, tag=f"v{j}", name=f"v{j}"))

        rqT_sb = [cpool.tile([128, S], BF, tag=f"rqT{hh}", name=f"rqT{hh}")
                  for hh in range(HPC)]
        oT_sb = [cpool.tile([128, S], BF, tag=f"oT{hh}", name=f"oT{hh}")
                 for hh in range(HPC)]

        HALF = [(0, 7), (7, 7)]
        ssq, ssk = [], []
        for g in range(2):
            tq = cpool.tile([128, 7], F32, tag=f"ssq{g}", name=f"ssq{g}")
            tk = cpool.tile([128, 7], F32, tag=f"ssk{g}", name=f"ssk{g}")
            nc.gpsimd.memset(tq[:], 0.0)
            nc.gpsimd.memset(tk[:], 0.0)
            ssq.append(tq)
            ssk.append(tk)

        # ---- phase 1: fused QKV projection; ss AllReduce per half ----
        eps_ap = cpool.tile([128, 1], F32, tag="eps_ap")
        nc.gpsimd.memset(eps_ap[:], EPS)
        rs_q, rs_ks = [], []

        def issue_ar(g):
            # staging on the gpsimd queue: naturally ordered just before the
            # collective trigger, immune to sync-queue scheduling shuffles
            nc.gpsimd.dma_start(out=ss_in[g][0], in_=ssq[g][:])
            nc.gpsimd.dma_start(out=ss_in[g][1], in_=ssk[g][:])
            nc.gpsimd.collective_compute(
                "AllReduce", mybir.AluOpType.add, replica_groups=[core_ids],
                ins=[ss_in[g][:]], outs=[ss_out[g][:]])
            ssg = cpool.tile([128, 14], F32, tag=f"ssg{g}", name=f"ssg{g}")
            nc.gpsimd.dma_start(out=ssg[:, 0:7], in_=ss_out[g][0])
            nc.gpsimd.dma_start(out=ssg[:, 7:14], in_=ss_out[g][1])
            return ssg

        ssg_bufs = {}

        def finish_ar(g):
            ssg = ssg_bufs[g]
            tmp = wpool.tile([128, 14], F32, tag="rstmp", name=f"rstmp{g}")
            nc.scalar.activation(tmp[:], ssg[:], A.Ln, scale=1.0 / DIM,
                                 bias=eps_ap[:])
            rqk = cpool.tile([128, 14], F32, tag=f"rqk{g}", name=f"rqk{g}")
            nc.scalar.activation(rqk[:], tmp[:], A.Exp, scale=-0.5)
            rs_q.append(rqk[:, 0:7])
            # k is NOT rms-scaled before rope (rope is linear); instead the
            # per-token k-scale folds into the exp scale AP of the n-tiles
            rsk = cpool.tile([128, 7], F32, tag=f"rsk{g}", name=f"rsk{g}")
            nc.scalar.mul(rsk[:, :], rqk[:, 7:14], SCALE)
            rs_ks.append(rsk)

        rq_store = {}
        rope_tr_pool = []

        def rope_dve_one(j, qi):
            off, sz = S_TILES[j]
            frt = wpool.tile([128, 128], BF, tag="frt", bufs=2,
                             name=f"frt{j}_{qi}")
            fit = wpool.tile([128, 128], BF, tag="fit", bufs=2,
                             name=f"fit{j}_{qi}")
            nc.sync.dma_start(out=frt[:sz, :], in_=fr2d[off:off + sz, :])
            nc.sync.dma_start(out=fit[:sz, :], in_=fi2d[off:off + sz, :])
            g, col = (0, j) if j < 7 else (1, j - 7)
            for qk, gB in ((q_sb[j], gqB), (k_sb[j], gkB))[qi:qi + 1]:
                qn = wpool.tile([128, CH], BF, tag="qn")
                if qi == 0:
                    nc.vector.scalar_tensor_tensor(
                        qn[:sz, :], qk[:sz, :], rs_q[g][:sz, col:col + 1],
                        gB[:sz, :], op0=Op.mult, op1=Op.mult)
                else:
                    nc.vector.tensor_mul(qn[:sz, :], qk[:sz, :], gB[:sz, :])
                q3 = qn[:sz, :].rearrange("p (h c) -> p h c", h=HPC)
                f3r = frt[:sz, :].rearrange("p (h c) -> p h c", h=HPC)
                f3i = fit[:sz, :].rearrange("p (h c) -> p h c", h=HPC)
                qe, qo = q3[:, :, 0:64], q3[:, :, 64:128]
                rq = ppool.tile([128, CH], BF, tag=f"rq{j}_{qi}",
                                name=f"rq{j}_{qi}")
                r3 = rq[:sz, :].rearrange("p (h c) -> p h c", h=HPC)
                t1 = wpool.tile([128, 128], BF, tag="ropet1")
                t2 = wpool.tile([128, 128], BF, tag="ropet2")
                t13 = t1[:sz, :].rearrange("p (h c) -> p h c", h=HPC)
                t23 = t2[:sz, :].rearrange("p (h c) -> p h c", h=HPC)
                nc.vector.tensor_mul(t13, qe, f3r)
                nc.vector.tensor_mul(t23, qo, f3i)
                nc.vector.tensor_sub(r3[:, :, 0:64], t13, t23)
                t3 = wpool.tile([128, 128], BF, tag="ropet1")
                t4 = wpool.tile([128, 128], BF, tag="ropet2")
                t33 = t3[:sz, :].rearrange("p (h c) -> p h c", h=HPC)
                t43 = t4[:sz, :].rearrange("p (h c) -> p h c", h=HPC)
                nc.vector.tensor_mul(t33, qe, f3i)
                nc.vector.tensor_mul(t43, qo, f3r)
                nc.vector.tensor_add(r3[:, :, 64:128], t33, t43)
                rq_store[(j, qi)] = rq

        def rope_tr_one(j, qi, pool, tag="tr"):
            off, sz = S_TILES[j]
            dstT, dcol = ((rqT_sb, 0), (kwT_sb, S))[qi]
            rq = rq_store[(j, qi)]
            for hh in range(HPC):
                tp = pool.tile([128, 128], BF, tag=tag)
                nc.tensor.transpose(tp[:, :sz], rq[:sz, D * hh:D * (hh + 1)],
                                    ident[:sz, :sz])
                nc.vector.tensor_copy(
                    dstT[hh][:, dcol + off:dcol + off + sz], tp[:, :sz])

        with tc.tile_pool(name="pj", bufs=2, space="PSUM") as pj:
            # HAM warm-up: the first ~14us are DMA-wait idle, after which the
            # projection would run at the cold 1.2 GHz clock for ~25us.  Keep
            # the PE array busy with identity matmuls (no input deps) so the
            # clock gate releases before the first real matmul.
            warm = pj.tile([128, 128], F32, tag="warm", bufs=1)

            def warm_burst(n):
                for _ in range(n):
                    nc.tensor.matmul(warm[:, :], ident[:, :], ident[:, :],
                                     start=True, stop=True)

            warm_burst(88)
            for j, (off, sz) in enumerate(S_TILES):
                if 1 <= j <= 4:
                    # bridge the early x-tile DMA waits so the clock gate
                    # stays released until the pipeline is flowing
                    warm_burst(20)
                if j == 0:
                    xt = xt0
                else:
                    xt = wpool.tile([128, DIM], BF, tag=f"xt{j % 2}", bufs=1,
                                    name=f"xt{j}")
                    nc.sync.dma_start(out=xt[:], in_=xT[j])
                ps = pj.tile([128, 512], F32, tag="qk")
                for kk in range(16):
                    nc.tensor.matmul(ps[:sz, 0:512], xt[:, 128 * kk:128 * kk + sz],
                                     wT_sb[kk][:, 0:512],
                                     start=(kk == 0), stop=False)
                nc.tensor.matmul(ps[:sz, 0:CH], ones128[0:1, :sz], bqR[0:1, :],
                                 start=False, stop=False, skip_group_check=True)
                nc.tensor.matmul(ps[:sz, CH:2 * CH], ones128[0:1, :sz],
                                 bkR[0:1, :], start=False, stop=True,
                                 skip_group_check=True)
                nc.scalar.copy(q_sb[j][:sz, :], ps[:sz, 0:CH])
                nc.scalar.copy(k_sb[j][:sz, :], ps[:sz, CH:2 * CH])
                g, col = (0, j) if j < 7 else (1, j - 7)
                sq = wpool.tile([128, CH], F32, tag="sqscratch")
                nc.scalar.activation(sq[:sz, :], q_sb[j][:sz, :], A.Square,
                                     accum_out=ssq[g][:sz, col:col + 1])
                sq2 = wpool.tile([128, CH], F32, tag="sqscratch")
                nc.scalar.activation(sq2[:sz, :], k_sb[j][:sz, :], A.Square,
                                     accum_out=ssk[g][:sz, col:col + 1])
                if j == 6:
                    ssg_bufs[0] = issue_ar(0)
            ssg_bufs[1] = issue_ar(1)
            for j, (off, sz) in enumerate(S_TILES):
                xt = wpool.tile([128, DIM], BF, tag=f"xtv{j % 2}", bufs=1,
                                name=f"xtv{j}")
                nc.sync.dma_start(out=xt[:], in_=xT[j])
                # k-cache / v-cache loads interleaved into the DMA stream so
                # they are resident before the first attention tiles
                if j in (2, 3):
                    nc.sync.dma_start(out=kwT_sb[j - 2][:, 0:S], in_=ckT[j - 2])
                if j in (4, 5):
                    big = cpool.tile([128, NT * D], BF, tag=f"cva{j - 4}",
                                     name=f"cva{j - 4}")
                    nc.sync.dma_start(out=big[:], in_=cv[j - 4])
                    cv_sb[j - 4] = [big[:, jj * D:(jj + 1) * D]
                                    for jj in range(NT)]
                ps = pj.tile([128, CH], F32, tag="v")
                for kk in range(16):
                    nc.tensor.matmul(ps[:sz, :], xt[:, 128 * kk:128 * kk + sz],
                                     wT_sb[kk][:, 512:768],
                                     start=(kk == 0), stop=False)
                nc.tensor.matmul(ps[:sz, :], ones128[0:1, :sz], bvR[0:1, :],
                                 start=False, stop=True, skip_group_check=True)
                nc.scalar.copy(v_sb[j][:sz, :], ps[:sz, :])
                if j == 1:
                    # k-rope no longer needs the AllReduce: run it on the
                    # otherwise-idle DVE during the v projection
                    for jk in range(NT):
                        rope_dve_one(jk, 1)
                if j == 2:
                    # q-rope for the first s-half: DVE is otherwise idle, so
                    # it runs as soon as the AllReduce result lands
                    finish_ar(0)
                    for jq in range(7):
                        rope_dve_one(jq, 0)
            # q-transposes after the v-loop (the first chunk's cache tiles
            # need only these); k-transposes overlap the c-tile exps later
            for jq in range(7):
                rope_tr_one(jq, 0, pj, tag="qk")

        woT_sb = []

        def load_woT():
            tpool = tc.alloc_tile_pool(name="tailp", bufs=1)
            for kk in range(16):
                t = tpool.tile([128, DIM], BF, tag=f"woTf{kk}", name=f"woTf{kk}")
                nc.sync.dma_start(out=t[:], in_=woT[128 * kk:128 * (kk + 1), :])
                woT_sb.append(t)
            return tpool

        # ---- phase 2 + 3: transposes interleaved with attention ----
        with tc.tile_pool(name="pat", bufs=2, space="PSUM") as pat:
            rope_tr_pool.append(pat)
            att = {}

            def attn_tiles(hh, jc, tlist):
                jof, jsz = SJ[jc]
                st = att.get((hh, jc))
                if st is None:
                    o_ps = pat.tile([128, 880], F32, tag="o", bufs=1,
                                    name=f"o{hh}_{jc}")
                    den = wpool.tile([128, 880], BF, tag="den", bufs=2,
                                     name=f"den{hh}_{jc}")
                    st = att[(hh, jc)] = (o_ps, den)
                o_ps, den = st
                for ti in tlist:
                    part, j2, toff, tsz = T_TILES[ti]
                    sc = pat.tile([128, 880], F32, tag="sc")
                    nc.tensor.matmul(
                        sc[:tsz, 0:512], kwT_sb[hh][:, toff:toff + tsz],
                        rqT_sb[hh][:, jof:jof + 512], start=True, stop=True)
                    nc.tensor.matmul(
                        sc[:tsz, 512:880], kwT_sb[hh][:, toff:toff + tsz],
                        rqT_sb[hh][:, jof + 512:jof + 880],
                        start=True, stop=True)
                    pT = wpool.tile([128, 880], BF, tag="pT", bufs=4)
                    if part == "c":
                        nc.scalar.activation(pT[:tsz, :], sc[:tsz, :], A.Exp,
                                             scale=SCALE)
                    else:
                        g2, col2 = (0, j2) if j2 < 7 else (1, j2 - 7)
                        nc.scalar.activation(
                            pT[:tsz, :], sc[:tsz, :], A.Exp,
                            scale=rs_ks[g2][:tsz, col2:col2 + 1])
                    if ti == 0:
                        nc.vector.tensor_copy(den[:, :], pT[:, :])
                    else:
                        nc.vector.tensor_add(den[:tsz, :], den[:tsz, :],
                                             pT[:tsz, :])
                    vt = (cv_sb[hh][j2][:tsz, :] if part == "c"
                          else v_sb[j2][:tsz, D * hh:D * (hh + 1)])
                    last = ti == len(T_TILES) - 1
                    nc.tensor.matmul(o_ps[:, 0:512], vt, pT[:tsz, 0:512],
                                     start=(ti == 0), stop=last)
                    nc.tensor.matmul(o_ps[:, 512:880], vt, pT[:tsz, 512:880],
                                     start=(ti == 0), stop=last)

            def attn_finish(hh, jc):
                # denominator column-sum on PE (ones-vector matmul), fast
                # reciprocal of the [1,880] row on DVE, partition-broadcast
                # on GPSIMD, multiply on DVE.  No ACT table switches, no PE
                # broadcast, and the sc-tag PSUM slot is released right after
                # the reciprocal.
                jof, jsz = SJ[jc]
                o_ps, den = att[(hh, jc)]
                dsum = pat.tile([128, 880], F32, tag="sc",
                                name=f"dsum{hh}_{jc}")
                nc.tensor.matmul(dsum[0:1, 0:512], ones_col[:, 0:1],
                                 den[:, 0:512], start=True, stop=True)
                nc.tensor.matmul(dsum[0:1, 512:880], ones_col[:, 0:1],
                                 den[:, 512:880], start=True, stop=True)
                rrow = wpool.tile([1, 880], F32, tag="rrow", bufs=2,
                                  name=f"rrow{hh}_{jc}")
                nc.vector.reciprocal_approx_fast(rrow[0:1, :jsz],
                                                 dsum[0:1, :jsz])
                denr = wpool.tile([128, 880], F32, tag="denr", bufs=2,
                                  name=f"denr{hh}_{jc}")
                nc.gpsimd.partition_broadcast(denr[:, :jsz], rrow[0:1, :jsz])
                nc.vector.tensor_mul(
                    oT_sb[hh][:, jof:jof + jsz], o_ps[:, :jsz], denr[:, :jsz])

            def emit_a2a(w, hh, split=False):
                # one contiguous store per destination core; for the final
                # chunk ACT is idle, so half the stores go to its queue
                for d_ in range(NCORES):
                    eng = nc.scalar if (split and d_ % 2) else nc.gpsimd
                    eng.dma_start(
                        out=a2a_in[w][hh][d_],
                        in_=oT_sb[hh][:, 880 * w + 110 * d_:
                                      880 * w + 110 * (d_ + 1)])
                nc.gpsimd.collective_compute(
                    "AllToAll", mybir.AluOpType.bypass,
                    replica_groups=[core_ids],
                    ins=[a2a_in[w][hh][:]], outs=[a2a_out[w][hh][:]])

            ywork = {}

            def _y_halfblock(w, n, hh):
                otr, yps, yf = ywork[w]
                yp = yps[n]
                for s8 in range(8):
                    kk = 8 * hh + s8
                    nc.tensor.matmul(
                        yp[:110, :],
                        otr[hh][:, 110 * s8:110 * (s8 + 1)],
                        woT_sb[2 * s8 + hh][:, 512 * n:512 * (n + 1)],
                        start=(kk == 0), stop=(kk == 15))
                if hh == 1:
                    nc.scalar.copy(yf[:110, 512 * n:512 * (n + 1)],
                                   yp[:110, :])
                    nc.sync.dma_start(
                        out=y_out[110 * w:110 * (w + 1),
                                  512 * n:512 * (n + 1)],
                        in_=yf[:110, 512 * n:512 * (n + 1)])

            def _otr_load(w, hh):
                t = tpool.tile([128, 8 * 110], BF, tag=f"otr{w}_{hh}",
                               name=f"otr{w}_{hh}")
                for d_ in range(NCORES):
                    nc.sync.dma_start(out=t[:, 110 * d_:110 * (d_ + 1)],
                                      in_=a2a_out[w][hh][d_])
                ywork[w][0][hh] = t

            def wave_y_h0(w):
                # Emitted BEFORE the head-1 AllToAll is triggered: the CC
                # core runs collectives in FIFO order and Tile serializes
                # later-emitted DRAM reads behind every prior collective,
                # so this half must precede the next collective emission.
                yf = wpool.tile([128, DIM], F32, tag="yf", bufs=1,
                                name=f"yf{w}")
                yps = [None] * 4
                for n in range(4 if w == 1 else 2):
                    if w == 1 and n >= 2:
                        # final wave: sc-tag PSUM is idle; 4 real buffers so
                        # every head-0 half pre-runs during the last AllToAll
                        t_ = pat.tile([128, 880], F32, tag="sc",
                                      name=f"yp{w}_{n}")
                        yps[n] = t_[:, 0:512]
                    else:
                        t_ = pat.tile([128, 512], F32, tag="tr",
                                      name=f"yp{w}_{n}")
                        yps[n] = t_[:, :]
                ywork[w] = ([None, None], yps, yf)
                _otr_load(w, 0)
                for n in range(4 if w == 1 else 2):
                    _y_halfblock(w, n, 0)

            def wave_y_h1a(w):
                _otr_load(w, 1)
                for n in range(2):
                    _y_halfblock(w, n, 1)

            def wave_y_h1b(w):
                yps = ywork[w][1]
                for n in (2, 3):
                    if yps[n] is None:
                        # lazy alloc: tr-tag rotation must observe the
                        # n-2 block's copy before reusing its buffer
                        t_ = pat.tile([128, 512], F32, tag="tr",
                                      name=f"yp{w}_{n}")
                        yps[n] = t_[:, :]
                        _y_halfblock(w, n, 0)
                    _y_halfblock(w, n, 1)

            finish_ar(1)
            attn_tiles(0, 0, range(0, 14))
            for jk in range(NT):
                rope_tr_one(jk, 1, pat)
            attn_tiles(0, 0, range(14, 28))
            attn_finish(0, 0)
            emit_a2a(0, 0)
            attn_tiles(1, 0, range(0, 14))
            for j in range(7, NT):
                rope_dve_one(j, 0)
                rope_tr_one(j, 0, pat)
            attn_tiles(1, 0, range(14, 28))
            attn_finish(1, 0)
            ppool.release()
            tpool = load_woT()
            attn_tiles(0, 1, range(0, 8))
            wave_y_h0(0)
            emit_a2a(0, 1)
            attn_tiles(0, 1, range(8, 14))
            attn_tiles(0, 1, range(14, 22))
            wave_y_h1a(0)
            attn_tiles(0, 1, range(22, 28))
            attn_finish(0, 1)
            emit_a2a(1, 0)
            attn_tiles(1, 1, range(0, 8))
            wave_y_h1b(0)
            attn_tiles(1, 1, range(8, 28))
            attn_finish(1, 1)
            wave_y_h0(1)
            emit_a2a(1, 1, split=True)
            wave_y_h1a(1)
            wave_y_h1b(1)
        tpool.release()


def _build():
    if "nc" not in _CACHE:
        nc = bacc.Bacc("TRN2", target_bir_lowering=False, debug=False,
                       num_devices=NCORES)
        _emit(nc)
        nc.compile()
        _CACHE["nc"] = nc
    return _CACHE["nc"]


def _make_fcomb(freqs):
    F, H, W = 2, 20, 44
    fr = np.asarray(freqs, np.float32)  # [1024, 64, 2]
    fpart = np.broadcast_to(fr[5:7, None, None, 0:22], (F, H, W, 22, 2))
    hpart = np.broadcast_to(fr[None, 0:H, None, 22:43], (F, H, W, 21, 2))
    wpart = np.broadcast_to(fr[None, None, 0:W, 43:64], (F, H, W, 21, 2))
    return np.concatenate([fpart, hpart, wpart], axis=3).reshape(S, 64, 2)


def kernel(x, wq, bq, wk, bk, wv, bv, wo, bo, gq, gk, freqs, cache_k, cache_v):
    x = np.asarray(x, np.float32)
    wq, wk, wv, wo = (np.asarray(a, np.float32) for a in (wq, wk, wv, wo))
    bq, bk, bv, bo = (np.asarray(a, np.float32) for a in (bq, bk, bv, bo))
    gq, gk = np.asarray(gq, np.float32), np.asarray(gk, np.float32)
    cache_k = np.asarray(cache_k, np.float32)
    cache_v = np.asarray(cache_v, np.float32)

    fcomb = _make_fcomb(freqs)
    fr2 = np.ascontiguousarray(np.tile(fcomb[..., 0], (1, HPC))).astype(BF16)
    fi2 = np.ascontiguousarray(np.tile(fcomb[..., 1], (1, HPC))).astype(BF16)
    # pre-tiled x^T: xT[j, p, kk*128+c] = x[128j+c, 128kk+p]
    xp = np.zeros((NT * 128, DIM), np.float32)
    xp[:S] = x[0]
    xT = np.ascontiguousarray(
        xp.reshape(NT, 128, 16, 128).transpose(0, 3, 2, 1).reshape(NT, 128, DIM)
    ).astype(BF16)

    # de-interleave rope channel pairs within each head: [2c] then [2c+1]
    # (applied consistently to wq/wk rows, their biases/gains, and the
    # transposed k-cache, so attention dot products are unchanged)
    perm = np.concatenate([np.arange(0, D, 2), np.arange(1, D, 2)])
    qk_perm = np.concatenate([h * D + perm for h in range(NH)])
    wqp, wkp = wq[qk_perm], wk[qk_perm]
    bqp, bkp = bq[qk_perm], bk[qk_perm]
    gqp, gkp = gq[qk_perm], gk[qk_perm]
    ck_perm = cache_k[0, WIN0:WIN0 + S][:, :, perm]  # [S, NH, D] channel-permuted

    woT_full = np.ascontiguousarray(wo.T).astype(BF16)  # [DIM, DIM]
    in_maps = []
    for c in range(NCORES):
        hs = slice(CH * c, CH * (c + 1))
        h0 = HPC * c
        wT = np.concatenate([wqp[hs].T, wkp[hs].T, wv[hs].T], axis=1).astype(BF16)
        woTc = woT_full
        ckTc = np.ascontiguousarray(
            ck_perm[:, h0:h0 + HPC, :].transpose(1, 2, 0)
        ).astype(BF16)  # [HPC, D, S]
        # pre-tiled cache-v: cvc[hh, p, j*128+d] = cv_window[128j+p, h, d]
        cw = np.zeros((NT * 128, HPC, D), np.float32)
        cw[:S] = cache_v[0, WIN0:WIN0 + S, h0:h0 + HPC, :]
        cvc = np.ascontiguousarray(
            cw.reshape(NT, 128, HPC, D).transpose(2, 1, 0, 3).reshape(HPC, 128, NT * D)
        ).astype(BF16)
        in_maps.append({
            "xT": xT, "wT": np.ascontiguousarray(wT), "woT": woTc,
            "ckT": ckTc, "cv": cvc, "fr2": fr2, "fi2": fi2,
            "gq": np.ascontiguousarray(gqp[hs])[None, :],
            "gk": np.ascontiguousarray(gkp[hs])[None, :],
            "bq": np.ascontiguousarray(bqp[hs])[None, :],
            "bk": np.ascontiguousarray(bkp[hs])[None, :],
            "bv": np.ascontiguousarray(bv[hs])[None, :],
        })

    nc = _build()
    res = run_bass_kernel_spmd(nc, in_maps, list(range(NCORES)))
    _CACHE["last_result"] = res
    # all-to-all layout: core c returns rows [110c:110c+110] and
    # [880+110c:880+110c+110]
    y = np.empty((S, DIM), np.float32)
    for c in range(NCORES):
        yc = res.results[c]["y"]
        y[110 * c:110 * (c + 1)] = yc[:110]
        y[880 + 110 * c:880 + 110 * (c + 1)] = yc[110:]
    return (y + bo[None, :]).reshape(1, S, DIM).astype(np.float32)



# revision 39
# speedup vs baseline: 1.0257x; 1.0257x over previous
"""Trainium2 Bass kernel for nn_CausalWanSelfAttention (sparse_attention).

Strategy: tensor-parallel over heads across 8 NeuronCores. Each core owns
2 of the 16 heads and processes all 1760 tokens:
  - fused QKV projection (bf16 matmuls, PSUM-accumulated over 16 k-tiles)
  - RMS-norm: local sum-of-squares, tiny AllReduce for the full-2048-channel
    statistic, ln/exp-based rsqrt on ACT
  - rope on DVE (free-dim channel pairs), PE-transpose of q/k per head
  - attention in transposed layout: scores^T = kw^T(T) @ rq^T, exp on ACT,
    PV accumulation on PE; softmax denominator via DVE accumulation +
    GPSIMD partition_all_reduce; normalize with a DVE divide
  - output projection, f32 ReduceScatter so each core emits 220 rows
Host side (free): input slicing/transposition/bf16 casts, rope freq table,
final concat + output bias.
"""
import sys

for _p in ("/opt/trn_rl_repo", "/root/.axon_site/_ro/trn_rl_repo"):
    if _p not in sys.path:
        sys.path.append(_p)

import numpy as np
import ml_dtypes

import concourse.bass as bass
import concourse.bacc as bacc
import concourse.mybir as mybir
from concourse import bass_isa
from concourse.tile import TileContext
from concourse.bass_utils import run_bass_kernel_spmd
from concourse.masks import make_identity

BF16 = ml_dtypes.bfloat16
S, DIM, NH, D = 1760, 2048, 16, 128
TW = 3520          # attention window length
WIN0 = 2640        # cache rows [2640:4400] form the first half of the window
NCORES, HPC = 8, 2
CH = HPC * D       # 256 channels per core
EPS = 1e-6
SCALE = 1.0 / float(np.sqrt(D))
S_OUT = S // NCORES  # 220 rows of output per core

S_TILES = [(i * 128, min(128, S - i * 128)) for i in range((S + 127) // 128)]
NT = len(S_TILES)  # 14
# window t-tiles: cache part [0,1760) then new part [1760,3520)
T_TILES = ([("c", j, off, sz) for j, (off, sz) in enumerate(S_TILES)]
           + [("n", j, off + S, sz) for j, (off, sz) in enumerate(S_TILES)])
SJ = [(0, 880), (880, 880)]  # attention s-chunks

_CACHE = {}


def _emit(nc):
    dt = mybir.dt
    BF, F32 = dt.bfloat16, dt.float32
    A = mybir.ActivationFunctionType
    Op = mybir.AluOpType
    core_ids = list(range(NCORES))

    xT = nc.declare_dram_parameter("xT", [NT, 128, DIM], BF, isOutput=False)
    wT = nc.declare_dram_parameter("wT", [DIM, 3 * CH], BF, isOutput=False)
    woT = nc.declare_dram_parameter("woT", [DIM, DIM], BF, isOutput=False)
    ckT = nc.declare_dram_parameter("ckT", [HPC, D, S], BF, isOutput=False)
    cv = nc.declare_dram_parameter("cv", [HPC, 128, NT * D], BF, isOutput=False)
    fr2d = nc.declare_dram_parameter("fr2", [S, 2 * 64], BF, isOutput=False)
    fi2d = nc.declare_dram_parameter("fi2", [S, 2 * 64], BF, isOutput=False)
    gqd = nc.declare_dram_parameter("gq", [1, CH], F32, isOutput=False)
    gkd = nc.declare_dram_parameter("gk", [1, CH], F32, isOutput=False)
    bqd = nc.declare_dram_parameter("bq", [1, CH], F32, isOutput=False)
    bkd = nc.declare_dram_parameter("bk", [1, CH], F32, isOutput=False)
    bvd = nc.declare_dram_parameter("bv", [1, CH], F32, isOutput=False)
    y_out = nc.declare_dram_parameter("y", [S_OUT, DIM], F32, isOutput=True)

    ss_in = [nc.dram_tensor(f"ss_in{g}", [2, 128, 7], F32) for g in range(2)]
    ss_out = [nc.dram_tensor(f"ss_out{g}", [2, 128, 7], F32, addr_space="Shared")
              for g in range(2)]
    # o-matrix all-to-all: two waves (s 0:880 and 880:1760) x two heads;
    # each core ends up with o^T columns for its own 110-row slice
    a2a_in = [[nc.dram_tensor(f"a2a_in{w}_{h}", [NCORES, D, 110], BF)
               for h in range(2)] for w in range(2)]
    a2a_out = [[nc.dram_tensor(f"a2a_out{w}_{h}", [NCORES, D, 110], BF)
                for h in range(2)] for w in range(2)]

    from contextlib import ExitStack
    with TileContext(nc) as tc, ExitStack() as stack:
        cpool = stack.enter_context(tc.tile_pool(name="const", bufs=1))
        wpool = stack.enter_context(tc.tile_pool(name="work", bufs=3))
        ppool = tc.alloc_tile_pool(name="projp", bufs=1)

        # ---- constants ----
        ident = cpool.tile([128, 128], BF, tag="ident")
        make_identity(nc, ident[:])
        ones_col = cpool.tile([128, 1], BF, tag="ones_col")
        nc.gpsimd.memset(ones_col[:], 1.0)
        ones128 = cpool.tile([1, 128], BF, tag="ones128")
        nc.gpsimd.memset(ones128[:], 1.0)

        xt0 = wpool.tile([128, DIM], BF, tag="xt0", bufs=1, name="xt0pre")
        nc.sync.dma_start(out=xt0[:], in_=xT[0])
        wT_sb = []
        for kk in range(16):
            t = ppool.tile([128, 3 * CH], BF, tag=f"wT{kk}", name=f"wT{kk}")
            eng = nc.sync if kk % 2 == 0 else nc.scalar
            eng.dma_start(out=t[:], in_=wT[128 * kk:128 * (kk + 1), :])
            wT_sb.append(t)

        def bcast_row(name, src):
            row = cpool.tile([1, CH], F32, tag=f"{name}_row", name=f"{name}_row")
            nc.sync.dma_start(out=row[:], in_=src[:])
            full = cpool.tile([128, CH], F32, tag=f"{name}_full", name=f"{name}_full")
            nc.gpsimd.partition_broadcast(full[:], row[:])
            return full

        def bias_row(name, src):
            # bf16 [1, CH] row used as the rhs of a rank-1 bias matmul
            row = cpool.tile([1, CH], F32, tag=f"{name}_row", name=f"{name}_row")
            nc.sync.dma_start(out=row[:], in_=src[:])
            rbf = cpool.tile([1, CH], BF, tag=f"{name}_bf", name=f"{name}_bf")
            nc.vector.tensor_copy(rbf[:], row[:])
            return rbf

        gqB = bcast_row("gq", gqd)
        gkB = bcast_row("gk", gkd)
        bqR = bias_row("bq", bqd)
        bkR = bias_row("bk", bkd)
        bvR = bias_row("bv", bvd)

        kwT_sb = []
        for hh in range(HPC):
            t = cpool.tile([128, TW], BF, tag=f"kwT{hh}", name=f"kwT{hh}")
            kwT_sb.append(t)
        cv_sb = [[], []]

        q_sb, k_sb, v_sb = [], [], []
        for j in range(NT):
            q_sb.append(ppool.tile([128, CH], F32, tag=f"q{j}", name=f"q{j}"))
            k_sb.append(ppool.tile([128, CH], F32, tag=f"k{j}", name=f"k{j}"))
            v_sb.append(cpool.tile([128, CH], BF, tag=f"v{j}", name=f"v{j}"))

        rqT_sb = [cpool.tile([128, S], BF, tag=f"rqT{hh}", name=f"rqT{hh}")
                  for hh in range(HPC)]
        oT_sb = [cpool.tile([128, S], BF, tag=f"oT{hh}", name=f"oT{hh}")
                 for hh in range(HPC)]

        HALF = [(0, 7), (7, 7)]
        ssq, ssk = [], []
        for g in range(2):
            tq = cpool.tile([128, 7], F32, tag=f"ssq{g}", name=f"ssq{g}")
            tk = cpool.tile([128, 7], F32, tag=f"ssk{g}", name=f"ssk{g}")
            nc.gpsimd.memset(tq[:], 0.0)
            nc.gpsimd.memset(tk[:], 0.0)
            ssq.append(tq)
            ssk.append(tk)

        # ---- phase 1: fused QKV projection; ss AllReduce per half ----
        eps_ap = cpool.tile([128, 1], F32, tag="eps_ap")
        nc.gpsimd.memset(eps_ap[:], EPS)
        rs_q, rs_ks = [], []

        def issue_ar(g):
            # staging on the gpsimd queue: naturally ordered just before the
            # collective trigger, immune to sync-queue scheduling shuffles
            nc.gpsimd.dma_start(out=ss_in[g][0], in_=ssq[g][:])
            nc.gpsimd.dma_start(out=ss_in[g][1], in_=ssk[g][:])
            nc.gpsimd.collective_compute(
                "AllReduce", mybir.AluOpType.add, replica_groups=[core_ids],
                ins=[ss_in[g][:]], outs=[ss_out[g][:]])
            ssg = cpool.tile([128, 14], F32, tag=f"ssg{g}", name=f"ssg{g}")
            nc.gpsimd.dma_start(out=ssg[:, 0:7], in_=ss_out[g][0])
            nc.gpsimd.dma_start(out=ssg[:, 7:14], in_=ss_out[g][1])
            return ssg

        ssg_bufs = {}

        def finish_ar(g):
            ssg = ssg_bufs[g]
            tmp = wpool.tile([128, 14], F32, tag="rstmp", name=f"rstmp{g}")
            nc.scalar.activation(tmp[:], ssg[:], A.Ln, scale=1.0 / DIM,
                                 bias=eps_ap[:])
            rqk = cpool.tile([128, 14], F32, tag=f"rqk{g}", name=f"rqk{g}")
            nc.scalar.activation(rqk[:], tmp[:], A.Exp, scale=-0.5)
            rs_q.append(rqk[:, 0:7])
            # k is NOT rms-scaled before rope (rope is linear); instead the
            # per-token k-scale folds into the exp scale AP of the n-tiles
            rsk = cpool.tile([128, 7], F32, tag=f"rsk{g}", name=f"rsk{g}")
            nc.scalar.mul(rsk[:, :], rqk[:, 7:14], SCALE)
            rs_ks.append(rsk)

        rq_store = {}
        rope_tr_pool = []

        def rope_dve_one(j, qi):
            off, sz = S_TILES[j]
            frt = wpool.tile([128, 128], BF, tag="frt", bufs=2,
                             name=f"frt{j}_{qi}")
            fit = wpool.tile([128, 128], BF, tag="fit", bufs=2,
                             name=f"fit{j}_{qi}")
            nc.sync.dma_start(out=frt[:sz, :], in_=fr2d[off:off + sz, :])
            nc.sync.dma_start(out=fit[:sz, :], in_=fi2d[off:off + sz, :])
            g, col = (0, j) if j < 7 else (1, j - 7)
            for qk, gB in ((q_sb[j], gqB), (k_sb[j], gkB))[qi:qi + 1]:
                qn = wpool.tile([128, CH], BF, tag="qn")
                if qi == 0:
                    nc.vector.scalar_tensor_tensor(
                        qn[:sz, :], qk[:sz, :], rs_q[g][:sz, col:col + 1],
                        gB[:sz, :], op0=Op.mult, op1=Op.mult)
                else:
                    nc.vector.tensor_mul(qn[:sz, :], qk[:sz, :], gB[:sz, :])
                q3 = qn[:sz, :].rearrange("p (h c) -> p h c", h=HPC)
                f3r = frt[:sz, :].rearrange("p (h c) -> p h c", h=HPC)
                f3i = fit[:sz, :].rearrange("p (h c) -> p h c", h=HPC)
                qe, qo = q3[:, :, 0:64], q3[:, :, 64:128]
                rq = ppool.tile([128, CH], BF, tag=f"rq{j}_{qi}",
                                name=f"rq{j}_{qi}")
                r3 = rq[:sz, :].rearrange("p (h c) -> p h c", h=HPC)
                t1 = wpool.tile([128, 128], BF, tag="ropet1")
                t2 = wpool.tile([128, 128], BF, tag="ropet2")
                t13 = t1[:sz, :].rearrange("p (h c) -> p h c", h=HPC)
                t23 = t2[:sz, :].rearrange("p (h c) -> p h c", h=HPC)
                nc.vector.tensor_mul(t13, qe, f3r)
                nc.vector.tensor_mul(t23, qo, f3i)
                nc.vector.tensor_sub(r3[:, :, 0:64], t13, t23)
                t3 = wpool.tile([128, 128], BF, tag="ropet1")
                t4 = wpool.tile([128, 128], BF, tag="ropet2")
                t33 = t3[:sz, :].rearrange("p (h c) -> p h c", h=HPC)
                t43 = t4[:sz, :].rearrange("p (h c) -> p h c", h=HPC)
                nc.vector.tensor_mul(t33, qe, f3i)
                nc.vector.tensor_mul(t43, qo, f3r)
                nc.vector.tensor_add(r3[:, :, 64:128], t33, t43)
                rq_store[(j, qi)] = rq

        def rope_tr_one(j, qi, pool, tag="tr"):
            off, sz = S_TILES[j]
            dstT, dcol = ((rqT_sb, 0), (kwT_sb, S))[qi]
            rq = rq_store[(j, qi)]
            for hh in range(HPC):
                tp = pool.tile([128, 128], BF, tag=tag)
                nc.tensor.transpose(tp[:, :sz], rq[:sz, D * hh:D * (hh + 1)],
                                    ident[:sz, :sz])
                nc.vector.tensor_copy(
                    dstT[hh][:, dcol + off:dcol + off + sz], tp[:, :sz])

        with tc.tile_pool(name="pj", bufs=2, space="PSUM") as pj:
            # HAM warm-up: the first ~14us are DMA-wait idle, after which the
            # projection would run at the cold 1.2 GHz clock for ~25us.  Keep
            # the PE array busy with identity matmuls (no input deps) so the
            # clock gate releases before the first real matmul.
            warm = pj.tile([128, 128], F32, tag="warm", bufs=1)

            def warm_burst(n):
                for _ in range(n):
                    nc.tensor.matmul(warm[:, :], ident[:, :], ident[:, :],
                                     start=True, stop=True)

            warm_burst(88)
            for j, (off, sz) in enumerate(S_TILES):
                if 1 <= j <= 4:
                    # bridge the early x-tile DMA waits so the clock gate
                    # stays released until the pipeline is flowing
                    warm_burst(20)
                if j == 0:
                    xt = xt0
                else:
                    xt = wpool.tile([128, DIM], BF, tag=f"xt{j % 2}", bufs=1,
                                    name=f"xt{j}")
                    nc.sync.dma_start(out=xt[:], in_=xT[j])
                ps = pj.tile([128, 512], F32, tag="qk")
                for kk in range(16):
                    nc.tensor.matmul(ps[:sz, 0:512], xt[:, 128 * kk:128 * kk + sz],
                                     wT_sb[kk][:, 0:512],
                                     start=(kk == 0), stop=False)
                nc.tensor.matmul(ps[:sz, 0:CH], ones128[0:1, :sz], bqR[0:1, :],
                                 start=False, stop=False, skip_group_check=True)
                nc.tensor.matmul(ps[:sz, CH:2 * CH], ones128[0:1, :sz],
                                 bkR[0:1, :], start=False, stop=True,
                                 skip_group_check=True)
                nc.scalar.copy(q_sb[j][:sz, :], ps[:sz, 0:CH])
                nc.scalar.copy(k_sb[j][:sz, :], ps[:sz, CH:2 * CH])
                g, col = (0, j) if j < 7 else (1, j - 7)
                sq = wpool.tile([128, CH], F32, tag="sqscratch")
                nc.scalar.activation(sq[:sz, :], q_sb[j][:sz, :], A.Square,
                                     accum_out=ssq[g][:sz, col:col + 1])
                sq2 = wpool.tile([128, CH], F32, tag="sqscratch")
                nc.scalar.activation(sq2[:sz, :], k_sb[j][:sz, :], A.Square,
                                     accum_out=ssk[g][:sz, col:col + 1])
                if j == 6:
                    ssg_bufs[0] = issue_ar(0)
            ssg_bufs[1] = issue_ar(1)
            for j, (off, sz) in enumerate(S_TILES):
                xt = wpool.tile([128, DIM], BF, tag=f"xtv{j % 2}", bufs=1,
                                name=f"xtv{j}")
                nc.sync.dma_start(out=xt[:], in_=xT[j])
                # k-cache / v-cache loads interleaved into the DMA stream so
                # they are resident before the first attention tiles
                if j in (2, 3):
                    nc.sync.dma_start(out=kwT_sb[j - 2][:, 0:S], in_=ckT[j - 2])
                if j in (4, 5):
                    big = cpool.tile([128, NT * D], BF, tag=f"cva{j - 4}",
                                     name=f"cva{j - 4}")
                    nc.sync.dma_start(out=big[:], in_=cv[j - 4])
                    cv_sb[j - 4] = [big[:, jj * D:(jj + 1) * D]
                                    for jj in range(NT)]
                ps = pj.tile([128, CH], F32, tag="v")
                for kk in range(16):
                    nc.tensor.matmul(ps[:sz, :], xt[:, 128 * kk:128 * kk + sz],
                                     wT_sb[kk][:, 512:768],
                                     start=(kk == 0), stop=False)
                nc.tensor.matmul(ps[:sz, :], ones128[0:1, :sz], bvR[0:1, :],
                                 start=False, stop=True, skip_group_check=True)
                nc.scalar.copy(v_sb[j][:sz, :], ps[:sz, :])
                if j == 1:
                    # k-rope no longer needs the AllReduce: run it on the
                    # otherwise-idle DVE during the v projection
                    for jk in range(NT):
                        rope_dve_one(jk, 1)
                if j == 2:
                    # q-rope for the first s-half: DVE is otherwise idle, so
                    # it runs as soon as the AllReduce result lands
                    finish_ar(0)
                    for jq in range(7):
                        rope_dve_one(jq, 0)
            # q-transposes after the v-loop (the first chunk's cache tiles
            # need only these); k-transposes overlap the c-tile exps later
            for jq in range(7):
                rope_tr_one(jq, 0, pj, tag="qk")

        woT_sb = []

        def load_woT():
            tpool = tc.alloc_tile_pool(name="tailp", bufs=1)
            for kk in range(16):
                t = tpool.tile([128, DIM], BF, tag=f"woTf{kk}", name=f"woTf{kk}")
                nc.sync.dma_start(out=t[:], in_=woT[128 * kk:128 * (kk + 1), :])
                woT_sb.append(t)
            return tpool

        # ---- phase 2 + 3: transposes interleaved with attention ----
        with tc.tile_pool(name="pat", bufs=2, space="PSUM") as pat:
            rope_tr_pool.append(pat)
            att = {}

            def attn_tiles(hh, jc, tlist):
                jof, jsz = SJ[jc]
                st = att.get((hh, jc))
                if st is None:
                    o_ps = pat.tile([128, 880], F32, tag="o", bufs=1,
                                    name=f"o{hh}_{jc}")
                    den = wpool.tile([128, 880], BF, tag="den", bufs=2,
                                     name=f"den{hh}_{jc}")
                    st = att[(hh, jc)] = (o_ps, den)
                o_ps, den = st
                for ti in tlist:
                    part, j2, toff, tsz = T_TILES[ti]
                    sc = pat.tile([128, 880], F32, tag="sc")
                    nc.tensor.matmul(
                        sc[:tsz, 0:512], kwT_sb[hh][:, toff:toff + tsz],
                        rqT_sb[hh][:, jof:jof + 512], start=True, stop=True)
                    nc.tensor.matmul(
                        sc[:tsz, 512:880], kwT_sb[hh][:, toff:toff + tsz],
                        rqT_sb[hh][:, jof + 512:jof + 880],
                        start=True, stop=True)
                    pT = wpool.tile([128, 880], BF, tag="pT", bufs=4)
                    if part == "c":
                        nc.scalar.activation(pT[:tsz, :], sc[:tsz, :], A.Exp,
                                             scale=SCALE)
                    else:
                        g2, col2 = (0, j2) if j2 < 7 else (1, j2 - 7)
                        nc.scalar.activation(
                            pT[:tsz, :], sc[:tsz, :], A.Exp,
                            scale=rs_ks[g2][:tsz, col2:col2 + 1])
                    if ti == 0:
                        nc.vector.tensor_copy(den[:, :], pT[:, :])
                    else:
                        nc.vector.tensor_add(den[:tsz, :], den[:tsz, :],
                                             pT[:tsz, :])
                    vt = (cv_sb[hh][j2][:tsz, :] if part == "c"
                          else v_sb[j2][:tsz, D * hh:D * (hh + 1)])
                    last = ti == len(T_TILES) - 1
                    nc.tensor.matmul(o_ps[:, 0:512], vt, pT[:tsz, 0:512],
                                     start=(ti == 0), stop=last)
                    nc.tensor.matmul(o_ps[:, 512:880], vt, pT[:tsz, 512:880],
                                     start=(ti == 0), stop=last)

            def attn_finish(hh, jc):
                # denominator column-sum on PE (ones-vector matmul), fast
                # reciprocal of the [1,880] row on DVE, partition-broadcast
                # on GPSIMD, multiply on DVE.  No ACT table switches, no PE
                # broadcast, and the sc-tag PSUM slot is released right after
                # the reciprocal.
                jof, jsz = SJ[jc]
                o_ps, den = att[(hh, jc)]
                dsum = pat.tile([128, 880], F32, tag="sc",
                                name=f"dsum{hh}_{jc}")
                nc.tensor.matmul(dsum[0:1, 0:512], ones_col[:, 0:1],
                                 den[:, 0:512], start=True, stop=True)
                nc.tensor.matmul(dsum[0:1, 512:880], ones_col[:, 0:1],
                                 den[:, 512:880], start=True, stop=True)
                rrow = wpool.tile([1, 880], F32, tag="rrow", bufs=2,
                                  name=f"rrow{hh}_{jc}")
                nc.vector.reciprocal_approx_fast(rrow[0:1, :jsz],
                                                 dsum[0:1, :jsz])
                denr = wpool.tile([128, 880], F32, tag="denr", bufs=2,
                                  name=f"denr{hh}_{jc}")
                nc.gpsimd.partition_broadcast(denr[:, :jsz], rrow[0:1, :jsz])
                nc.vector.tensor_mul(
                    oT_sb[hh][:, jof:jof + jsz], o_ps[:, :jsz], denr[:, :jsz])

            def emit_a2a(w, hh, split=False):
                # one contiguous store per destination core; for the final
                # chunk ACT is idle, so half the stores go to its queue
                for d_ in range(NCORES):
                    eng = nc.scalar if (split and d_ % 2) else nc.gpsimd
                    eng.dma_start(
                        out=a2a_in[w][hh][d_],
                        in_=oT_sb[hh][:, 880 * w + 110 * d_:
                                      880 * w + 110 * (d_ + 1)])
                nc.gpsimd.collective_compute(
                    "AllToAll", mybir.AluOpType.bypass,
                    replica_groups=[core_ids],
                    ins=[a2a_in[w][hh][:]], outs=[a2a_out[w][hh][:]])

            ywork = {}

            def _y_halfblock(w, n, hh):
                otr, yps, yf = ywork[w]
                yp = yps[n]
                for s8 in range(8):
                    kk = 8 * hh + s8
                    nc.tensor.matmul(
                        yp[:110, :],
                        otr[hh][:, 110 * s8:110 * (s8 + 1)],
                        woT_sb[2 * s8 + hh][:, 512 * n:512 * (n + 1)],
                        start=(kk == 0), stop=(kk == 15))
                if hh == 1:
                    nc.scalar.copy(yf[:110, 512 * n:512 * (n + 1)],
                                   yp[:110, :])
                    nc.sync.dma_start(
                        out=y_out[110 * w:110 * (w + 1),
                                  512 * n:512 * (n + 1)],
                        in_=yf[:110, 512 * n:512 * (n + 1)])

            def _otr_load(w, hh):
                t = tpool.tile([128, 8 * 110], BF, tag=f"otr{w}_{hh}",
                               name=f"otr{w}_{hh}")
                for d_ in range(NCORES):
                    nc.sync.dma_start(out=t[:, 110 * d_:110 * (d_ + 1)],
                                      in_=a2a_out[w][hh][d_])
                ywork[w][0][hh] = t

            def wave_y_h0(w):
                # Emitted BEFORE the head-1 AllToAll is triggered: the CC
                # core runs collectives in FIFO order and Tile serializes
                # later-emitted DRAM reads behind every prior collective,
                # so this half must precede the next collective emission.
                yf = wpool.tile([128, DIM], F32, tag="yf", bufs=1,
                                name=f"yf{w}")
                yps = [None] * 4
                for n in range(4 if w == 1 else 2):
                    if w == 1 and n >= 2:
                        # final wave: sc-tag PSUM is idle; 4 real buffers so
                        # every head-0 half pre-runs during the last AllToAll
                        t_ = pat.tile([128, 880], F32, tag="sc",
                                      name=f"yp{w}_{n}")
                        yps[n] = t_[:, 0:512]
                    else:
                        t_ = pat.tile([128, 512], F32, tag="tr",
                                      name=f"yp{w}_{n}")
                        yps[n] = t_[:, :]
                ywork[w] = ([None, None], yps, yf)
                _otr_load(w, 0)
                for n in range(4 if w == 1 else 2):
                    _y_halfblock(w, n, 0)

            def wave_y_h1a(w):
                _otr_load(w, 1)
                for n in range(2):
                    _y_halfblock(w, n, 1)

            def wave_y_h1b(w):
                yps = ywork[w][1]
                for n in (2, 3):
                    if yps[n] is None:
                        # lazy alloc: tr-tag rotation must observe the
                        # n-2 block's copy before reusing its buffer
                        t_ = pat.tile([128, 512], F32, tag="tr",
                                      name=f"yp{w}_{n}")
                        yps[n] = t_[:, :]
                        _y_halfblock(w, n, 0)
                    _y_halfblock(w, n, 1)

            attn_tiles(0, 0, range(0, 2))
            finish_ar(1)
            attn_tiles(0, 0, range(2, 14))
            for jk in range(NT):
                rope_tr_one(jk, 1, pat)
            attn_tiles(0, 0, range(14, 28))
            attn_finish(0, 0)
            emit_a2a(0, 0)
            attn_tiles(1, 0, range(0, 14))
            for j in range(7, NT):
                rope_dve_one(j, 0)
                rope_tr_one(j, 0, pat)
            attn_tiles(1, 0, range(14, 28))
            attn_finish(1, 0)
            ppool.release()
            tpool = load_woT()
            attn_tiles(0, 1, range(0, 8))
            wave_y_h0(0)
            emit_a2a(0, 1)
            attn_tiles(0, 1, range(8, 14))
            attn_tiles(0, 1, range(14, 22))
            wave_y_h1a(0)
            attn_tiles(0, 1, range(22, 28))
            attn_finish(0, 1)
            emit_a2a(1, 0)
            attn_tiles(1, 1, range(0, 8))
            wave_y_h1b(0)
            attn_tiles(1, 1, range(8, 28))
            attn_finish(1, 1)
            wave_y_h0(1)
            emit_a2a(1, 1, split=True)
            wave_y_h1a(1)
            wave_y_h1b(1)
        tpool.release()


def _build():
    if "nc" not in _CACHE:
        nc = bacc.Bacc("TRN2", target_bir_lowering=False, debug=False,
                       num_devices=NCORES)
        _emit(nc)
        nc.compile()
        _CACHE["nc"] = nc
    return _CACHE["nc"]


def _make_fcomb(freqs):
    F, H, W = 2, 20, 44
    fr = np.asarray(freqs, np.float32)  # [1024, 64, 2]
    fpart = np.broadcast_to(fr[5:7, None, None, 0:22], (F, H, W, 22, 2))
    hpart = np.broadcast_to(fr[None, 0:H, None, 22:43], (F, H, W, 21, 2))
    wpart = np.broadcast_to(fr[None, None, 0:W, 43:64], (F, H, W, 21, 2))
    return np.concatenate([fpart, hpart, wpart], axis=3).reshape(S, 64, 2)


def kernel(x, wq, bq, wk, bk, wv, bv, wo, bo, gq, gk, freqs, cache_k, cache_v):
    x = np.asarray(x, np.float32)
    wq, wk, wv, wo = (np.asarray(a, np.float32) for a in (wq, wk, wv, wo))
    bq, bk, bv, bo = (np.asarray(a, np.float32) for a in (bq, bk, bv, bo))
    gq, gk = np.asarray(gq, np.float32), np.asarray(gk, np.float32)
    cache_k = np.asarray(cache_k, np.float32)
    cache_v = np.asarray(cache_v, np.float32)

    fcomb = _make_fcomb(freqs)
    fr2 = np.ascontiguousarray(np.tile(fcomb[..., 0], (1, HPC))).astype(BF16)
    fi2 = np.ascontiguousarray(np.tile(fcomb[..., 1], (1, HPC))).astype(BF16)
    # pre-tiled x^T: xT[j, p, kk*128+c] = x[128j+c, 128kk+p]
    xp = np.zeros((NT * 128, DIM), np.float32)
    xp[:S] = x[0]
    xT = np.ascontiguousarray(
        xp.reshape(NT, 128, 16, 128).transpose(0, 3, 2, 1).reshape(NT, 128, DIM)
    ).astype(BF16)

    # de-interleave rope channel pairs within each head: [2c] then [2c+1]
    # (applied consistently to wq/wk rows, their biases/gains, and the
    # transposed k-cache, so attention dot products are unchanged)
    perm = np.concatenate([np.arange(0, D, 2), np.arange(1, D, 2)])
    qk_perm = np.concatenate([h * D + perm for h in range(NH)])
    wqp, wkp = wq[qk_perm], wk[qk_perm]
    bqp, bkp = bq[qk_perm], bk[qk_perm]
    gqp, gkp = gq[qk_perm], gk[qk_perm]
    ck_perm = cache_k[0, WIN0:WIN0 + S][:, :, perm]  # [S, NH, D] channel-permuted

    woT_full = np.ascontiguousarray(wo.T).astype(BF16)  # [DIM, DIM]
    in_maps = []
    for c in range(NCORES):
        hs = slice(CH * c, CH * (c + 1))
        h0 = HPC * c
        wT = np.concatenate([wqp[hs].T, wkp[hs].T, wv[hs].T], axis=1).astype(BF16)
        woTc = woT_full
        ckTc = np.ascontiguousarray(
            ck_perm[:, h0:h0 + HPC, :].transpose(1, 2, 0)
        ).astype(BF16)  # [HPC, D, S]
        # pre-tiled cache-v: cvc[hh, p, j*128+d] = cv_window[128j+p, h, d]
        cw = np.zeros((NT * 128, HPC, D), np.float32)
        cw[:S] = cache_v[0, WIN0:WIN0 + S, h0:h0 + HPC, :]
        cvc = np.ascontiguousarray(
            cw.reshape(NT, 128, HPC, D).transpose(2, 1, 0, 3).reshape(HPC, 128, NT * D)
        ).astype(BF16)
        in_maps.append({
            "xT": xT, "wT": np.ascontiguousarray(wT), "woT": woTc,
            "ckT": ckTc, "cv": cvc, "fr2": fr2, "fi2": fi2,
            "gq": np.ascontiguousarray(gqp[hs])[None, :],
            "gk": np.ascontiguousarray(gkp[hs])[None, :],
            "bq": np.ascontiguousarray(bqp[hs])[None, :],
            "bk": np.ascontiguousarray(bkp[hs])[None, :],
            "bv": np.ascontiguousarray(bv[hs])[None, :],
        })

    nc = _build()
    res = run_bass_kernel_spmd(nc, in_maps, list(range(NCORES)))
    _CACHE["last_result"] = res
    # all-to-all layout: core c returns rows [110c:110c+110] and
    # [880+110c:880+110c+110]
    y = np.empty((S, DIM), np.float32)
    for c in range(NCORES):
        yc = res.results[c]["y"]
        y[110 * c:110 * (c + 1)] = yc[:110]
        y[880 + 110 * c:880 + 110 * (c + 1)] = yc[110:]
    return (y + bo[None, :]).reshape(1, S, DIM).astype(np.float32)

